# revision 21
# baseline (speedup 1.0000x reference)
"""Trainium2 Bass kernel for nn_Loss_2 (weighted BCE + index-gathered CE mean).

Data-parallel over 8 NeuronCores: each core processes 8 of the 64 batches.
The comb stream is fp8-e4m3 (quarter of f32 HBM traffic), partial sums are
f32 in PSUM, host does the final f64 weighted reduction.

The comb stream carries 22 "classes" per token (class-major [P, 22, Tp]):
  classes 0..19 : max(y_pred_comb, 2^-9)        (fp8 clamp, keeps ln finite)
  class  20     : ys ? 1 : (1-ps)   -> ln = (1-ys)*ln(1-ps)
  class  21     : ys ? ps : 1       -> ln = ys*ln(ps)
Per tile (Tp tokens/partition):
  lnc  = Ln(comb_ext)                     (ScalarE, fp8 -> bf16, 1 pass)
  mask = (iota_c == idxg), classes 0..19  (DVE TT is_equal bf16, 2x mode)
  prod = mask * lnc[0:20]                 (DVE TT mult bf16, 2x mode)
  PSUM A += colsum(prod)                  (TensorE ones-matmul, 10 chunks)
  PSUM B += colsum(lnc[20]); C += colsum(lnc[21])   (TensorE, 1 chunk each)
with idxg = y_comb if ys==1 else 20 (never matches -> mask row 0).
Host: loss = -(sum(A) + W0*sum(B) + W1*sum(C)) / (B*S)
"""

import sys

if '/opt/trn_rl_repo' not in sys.path:
    sys.path.insert(0, '/opt/trn_rl_repo')

import numpy as np
import ml_dtypes

import concourse.bass as bass
import concourse.bacc as bacc
import concourse.tile as tile
import concourse.mybir as mybir
from concourse.bass_utils import run_bass_kernel_spmd

F32 = mybir.dt.float32
BF16 = mybir.dt.bfloat16
FP8 = mybir.dt.float8e4
BF16_NP = ml_dtypes.bfloat16
FP8_NP = ml_dtypes.float8_e4m3fn

B, S, C = 64, 16384, 20
CE = C + 2                      # extended classes: +(1-ps)-gated, +ps-gated
W0, W1 = 0.51, 19.05
P = 128
N_CORES = 8
Tp = 128                        # tokens per partition per tile
NT = (B // N_CORES) * S // (P * Tp)   # tiles per core
IW = 32                         # iota inner period (dense run length)
AUXW = C * IW + NT * Tp         # iota block + all idxg tiles
MM = 512                        # matmul moving-free chunk (= psum bank f32)
ALU = mybir.AluOpType
AF = mybir.ActivationFunctionType


def _build(NT, Tp):
    FREE = Tp * CE              # full extended width
    CW = Tp * C                 # comb-classes width
    nc = bacc.Bacc("TRN2", target_bir_lowering=False, debug=False)

    comb_d = nc.dram_tensor("comb", [NT, P, FREE], FP8, kind="ExternalInput").ap()
    aux_d = nc.dram_tensor("aux", [P, AUXW], BF16, kind="ExternalInput").ap()
    out_d = nc.dram_tensor("out", [1, MM + 2 * Tp], F32, kind="ExternalOutput").ap()

    with tile.TileContext(nc) as tc:
        with (
            tc.tile_pool(name="const", bufs=1) as const_pool,
            tc.tile_pool(name="comb", bufs=1) as comb_pool,
            tc.tile_pool(name="lnc", bufs=2) as lnc_pool,
            tc.tile_pool(name="mask", bufs=2) as mask_pool,
            tc.tile_pool(name="prod", bufs=2) as prod_pool,
            tc.tile_pool(name="psum", bufs=1,
                         space=bass.MemorySpace.PSUM) as psum_pool,
        ):
            # warm the natural_log activation table while first DMAs run
            warm = const_pool.tile([P, 1], BF16)
            nc.vector.memset(warm[:], 1.0)
            nc.scalar.activation(warm[:], warm[:], AF.Ln)

            ones = const_pool.tile([P, 1], BF16)
            nc.vector.memset(ones[:], 1.0)

            # comb0 leads the queue (it gates the ActE Ln chain) and is
            # split in half so the first Ln can start before the cold DMA
            # stream finishes ramping; then the small aux (iota + all
            # idxg), then the rest of the comb stream.
            comb_t0 = comb_pool.tile([P, FREE], FP8, tag="comb0")
            comb_ts = [comb_t0]
            H0 = (FREE // 2) // Tp * Tp
            nc.sync.dma_start(comb_t0[:, 0:H0], comb_d[0][:, 0:H0])
            nc.sync.dma_start(comb_t0[:, H0:FREE], comb_d[0][:, H0:FREE])

            aux_t = const_pool.tile([P, AUXW], BF16)
            nc.sync.dma_start(aux_t[:], aux_d[:])
            iota_v = aux_t[:, 0:C * IW].rearrange("p (c o t) -> p c o t",
                                                  c=C, o=1)

            for i in range(1, NT):
                comb_t = comb_pool.tile([P, FREE], FP8, tag=f"comb{i}")
                nc.sync.dma_start(comb_t[:], comb_d[i])
                comb_ts.append(comb_t)

            pA = psum_pool.tile([1, MM], F32, tag="pA")
            pBC = psum_pool.tile([1, 2 * Tp], F32, tag="pBC")

            NCH = CW // MM
            for i in range(NT):
                comb_t = comb_ts[i]
                off = C * IW + i * Tp
                idxg = aux_t[:, off:off + Tp]
                idxg_v = idxg.rearrange("p (o r t) -> p o r t", o=1, t=IW)

                lnc = lnc_pool.tile([P, FREE], BF16, tag="lnc")
                if i == 0:
                    # two half-passes: the first starts as soon as the
                    # first half-DMA of comb0 lands
                    nc.scalar.activation(lnc[:, 0:H0], comb_t[:, 0:H0],
                                         AF.Ln)
                    nc.scalar.activation(lnc[:, H0:FREE], comb_t[:, H0:FREE],
                                         AF.Ln)
                else:
                    nc.scalar.activation(lnc[:], comb_t[:], AF.Ln)

                mask = mask_pool.tile([P, CW], BF16, tag="mask")
                mask_v = mask[:].rearrange("p (c r t) -> p c r t", c=C, t=IW)
                b_iota, b_idxg = bass.broadcast_tensor_aps(iota_v, idxg_v)
                nc.vector.tensor_tensor(mask_v, b_iota, b_idxg, ALU.is_equal)

                prod = prod_pool.tile([P, CW], BF16, tag="prod")
                first, last = (i == 0), (i == NT - 1)
                if last:
                    # split the final multiply so TensorE can drain the
                    # first chunks while the second half still runs
                    HL = 3 * MM
                    nc.vector.tensor_tensor(prod[:, 0:HL], mask[:, 0:HL],
                                            lnc[:, 0:HL], ALU.mult)
                    nc.vector.tensor_tensor(prod[:, HL:CW], mask[:, HL:CW],
                                            lnc[:, HL:CW], ALU.mult)
                else:
                    nc.vector.tensor_tensor(prod[:], mask[:], lnc[:, 0:CW],
                                            ALU.mult)

                for c in range(NCH):
                    nc.tensor.matmul(pA[:], ones[:],
                                     prod[:, c * MM:(c + 1) * MM],
                                     start=(first and c == 0),
                                     stop=(last and c == NCH - 1))
                nc.tensor.matmul(pBC[:], ones[:], lnc[:, CW:FREE],
                                 start=first, stop=last)

            res_t = const_pool.tile([1, MM + 2 * Tp], F32)
            nc.scalar.copy(res_t[:, 0:MM], pA[:])
            nc.scalar.copy(res_t[:, MM:MM + 2 * Tp], pBC[:])
            nc.sync.dma_start(out_d[:], res_t[:])

    nc.compile()
    return nc


_NC_CACHE = {}


def make_in_maps(y_pred_stroke, y_pred_comb, y_stroke, y_comb):
    y_pred_stroke = np.asarray(y_pred_stroke, dtype=np.float32)
    y_pred_comb = np.asarray(y_pred_comb, dtype=np.float32)
    y_stroke = np.asarray(y_stroke, dtype=np.float32)
    y_comb = np.asarray(y_comb)
    FREE = Tp * CE
    Bc = B // N_CORES
    iota = np.repeat(np.arange(C, dtype=np.float32), IW)
    in_maps = []
    for c in range(N_CORES):
        sl = slice(c * Bc, (c + 1) * Bc)
        ys = np.ascontiguousarray(y_stroke[sl])[..., 0].reshape(-1)
        ps = np.ascontiguousarray(y_pred_stroke[sl])[..., 0].reshape(-1)
        yc = np.ascontiguousarray(y_comb[sl]).reshape(-1)
        pos = ys > 0.5
        comb = (np.maximum(np.ascontiguousarray(y_pred_comb[sl]), 2.0 ** -9)
                .reshape(NT, P, Tp, C)
                .transpose(0, 1, 3, 2))                     # [NT, P, C, Tp]
        q0 = np.where(pos, 1.0, 1.0 - ps).reshape(NT, P, 1, Tp)
        q1 = np.where(pos, ps, 1.0).reshape(NT, P, 1, Tp)
        comb_ext = np.concatenate([comb, q0, q1], axis=2).reshape(NT, P, FREE)
        idxg = np.where(pos, yc.astype(np.float32), 20.0)
        aux = np.empty((P, AUXW), dtype=np.float32)
        aux[:, 0:C * IW] = iota[None, :]
        aux[:, C * IW:] = (idxg.reshape(NT, P, Tp)
                           .transpose(1, 0, 2).reshape(P, NT * Tp))
        in_maps.append({
            "comb": np.ascontiguousarray(comb_ext).astype(FP8_NP),
            "aux": aux.astype(BF16_NP),
        })
    return in_maps


def kernel(y_pred_stroke, y_pred_comb, y_stroke, y_comb):
    key = (NT, Tp)
    if key not in _NC_CACHE:
        _NC_CACHE[key] = _build(NT, Tp)
    nc = _NC_CACHE[key]
    in_maps = make_in_maps(y_pred_stroke, y_pred_comb, y_stroke, y_comb)
    res = run_bass_kernel_spmd(nc, in_maps, list(range(N_CORES)))
    total = 0.0
    for r in res.results:
        o = r["out"].astype(np.float64).reshape(-1)
        total += (o[0:MM].sum() + W0 * o[MM:MM + Tp].sum()
                  + W1 * o[MM + Tp:].sum())
    return np.asarray([-total / (B * S)], dtype=np.float32)
